# revision 6
# baseline (speedup 1.0000x reference)
"""Trainium2 Bass kernel for nn_KVAttnDecoderRNN (GRU decoder step + dot attention
+ KB embedding branch + vocab projection), tensor-parallel over 8 NeuronCores.

Sharding:
  - GRU gates sharded over hidden (64 rows/core), h1 AllGather (tiny).
  - Attention sharded over L=512 (64 l/core); softmax over batch is local per l;
    context partials AllReduce (20KB).
  - Out projection sharded over vocab (4000 rows/core), host concat.
  - KB embedding gather sharded over the pad dim (64/core) via one dma_gather.
All per-core variation is carried by input DATA (sliced/transposed on host) so a
single SPMD program runs on all 8 cores.
"""

import math
import os
import sys

import numpy as np

for _p in ("/opt/trn_rl_repo", "/root/.axon_site/_ro/trn_rl_repo"):
    if _p not in sys.path:
        sys.path.append(_p)

B = 10
H = 512
VOCAB = 32000
KB = 431
KB_PAD = 1523
NCORES = 8
VS = VOCAB // NCORES  # 4000 vocab rows per core
LS = H // NCORES      # 64 attention positions per core
GS = H // NCORES      # 64 GRU gate rows per core
FLAT = KB * H         # 220672 flat e2 elements per batch
SLAB = FLAT // NCORES  # 27584 flat elements per core per batch
NJ = 64               # padded kb rows per (core, batch)
NIDX = 3 * B * NJ     # 1920 gather indices per core

_CACHE = {}


def _build_program():
    import concourse.bass as bass
    import concourse.tile as tile
    from concourse import bacc, mybir
    from concourse.masks import make_identity

    fp32 = mybir.dt.float32
    i16 = mybir.dt.int16
    AF = mybir.ActivationFunctionType

    nc = bacc.Bacc("TRN2", target_bir_lowering=False, debug=False,
                   enable_asserts=False, num_devices=NCORES)

    # ---- I/O ----
    def din(name, shape, dt=fp32):
        return nc.dram_tensor(name, list(shape), dt, kind="ExternalInput").ap()

    def dout(name, shape, dt=fp32):
        return nc.dram_tensor(name, list(shape), dt, kind="ExternalOutput").ap()

    xh = din("xh", (2, B, H))            # [emb rows x; h0]
    h0s = din("h0s", (B, GS))            # h0 slice for this core's gate rows
    wih = din("wih", (H, 3 * GS))        # w_ih[not gate rows].T  (cols r|z|n)
    whh = din("whh", (H, 3 * GS))
    bih = din("bih", (1, 3 * GS))
    bhh = din("bhh", (1, 3 * GS))
    encs = din("encs", (LS, B, H))
    wcat = din("wcat", (2 * H, H))       # w_concat.T
    bcat = din("bcat", (H,))
    wvo = din("wvo", (H, VS))            # w_out slab .T
    bvo = din("bvo", (1, VS))
    kbidx = din("kbidx", (128, NIDX // 16), i16)
    embkb = din("embkb", (VOCAB, H))

    logits = dout("logits", (B, VS))
    attnw = dout("attnw", (LS, B))
    h1T_o = dout("h1T", (H, B))
    ctx_o = dout("ctxo", (B, H))
    e2c = dout("e2c", (B * NJ, H))

    # internal DRAM for collectives
    h1cc = nc.dram_tensor("h1cc", [GS, B], fp32).ap()
    h1all = nc.dram_tensor("h1all", [H, B], fp32, addr_space="Shared").ap()
    ctxcc = nc.dram_tensor("ctxcc", [B, H], fp32).ap()
    ctxall = nc.dram_tensor("ctxall", [B, H], fp32, addr_space="Shared").ap()

    groups = [list(range(NCORES))]

    with tile.TileContext(nc) as tc, \
         tc.tile_pool(name="const", bufs=1) as constp, \
         tc.tile_pool(name="sb", bufs=1) as sb, \
         tc.tile_pool(name="psT", bufs=2, space="PSUM") as psT, \
         tc.tile_pool(name="psM", bufs=4, space="PSUM") as psM:

        ident = constp.tile([128, 128], fp32)
        make_identity(nc, ident[:])
        ones10 = constp.tile([1, B], fp32)
        nc.vector.memset(ones10[:], 1.0)

        def cp(use_vec, dst, src):
            if use_vec:
                nc.vector.tensor_copy(dst, src)
            else:
                nc.scalar.copy(dst, src)

        # ---------- phase 0: small input DMAs ----------
        xh_s = sb.tile([B, 2, H], fp32, tag="xh")
        nc.sync.dma_start(xh_s[:], xh.rearrange("t b h -> b t h"))
        h0s_s = sb.tile([B, GS], fp32, tag="h0s")
        nc.sync.dma_start(h0s_s[:], h0s[:])
        wih_s = sb.tile([128, 4, 3 * GS], fp32, tag="wih")
        nc.sync.dma_start(wih_s[:], wih.rearrange("(k p) r -> p k r", p=128))
        whh_s = sb.tile([128, 4, 3 * GS], fp32, tag="whh")
        nc.sync.dma_start(whh_s[:], whh.rearrange("(k p) r -> p k r", p=128))
        bih_s = sb.tile([1, 3 * GS], fp32, tag="bih")
        nc.sync.dma_start(bih_s[:], bih[:])
        bhh_s = sb.tile([1, 3 * GS], fp32, tag="bhh")
        nc.sync.dma_start(bhh_s[:], bhh[:])

        # kb gather kicked off early on gpsimd (independent branch)
        idx_s = sb.tile([128, NIDX // 16], i16, tag="idx")
        nc.sync.dma_start(idx_s[:], kbidx[:])
        kbg = sb.tile([128, 15, H], fp32, tag="kbg")
        if os.environ.get("K_NOGATHER"):
            nc.vector.memset(kbg[:], 0.25)
        else:
            nc.gpsimd.dma_gather(kbg[:], embkb[:], idx_s[:], NIDX, NIDX, H,
                                 single_packet=False)

        # big weight slabs
        E = sb.tile([LS, B * H], fp32, tag="E")
        nc.sync.dma_start(E[:], encs.rearrange("l b h -> l (b h)"))
        wcat_s = sb.tile([128, 8, H], fp32, tag="wcat")
        nc.sync.dma_start(wcat_s[:], wcat.rearrange("(k p) m -> p k m", p=128))
        bcat_s = sb.tile([128, 4], fp32, tag="bcat")
        nc.sync.dma_start(bcat_s[:], bcat.rearrange("(m p) -> p m", p=128))
        bvo_s = sb.tile([1, VS], fp32, tag="bvo")
        nc.sync.dma_start(bvo_s[:], bvo[:])
        wvo_r = wvo.rearrange("(k p) n -> p k n", p=128)
        wvo_t = []
        for ns in range(8):
            n0 = ns * 512
            n1 = min(VS, n0 + 512)
            t = sb.tile([128, 4, 512], fp32, tag=f"wvo{ns}")
            nc.sync.dma_start(t[:, :, : n1 - n0], wvo_r[:, :, n0:n1])
            wvo_t.append(t)

        # ---------- phase 1: transposes of x / h0 ----------
        xT = sb.tile([128, 4, B], fp32, tag="xT")
        h0T = sb.tile([128, 4, B], fp32, tag="h0T")
        for t, dst in ((0, xT), (1, h0T)):
            for j in range(4):
                ps = psT.tile([128, B], fp32, tag="t")
                nc.tensor.transpose(ps[:], xh_s[:, t, j * 128:(j + 1) * 128],
                                    ident[:B, :B])
                cp(j % 2, dst[:, j, :], ps[:])
        ps_h0s = psT.tile([GS, B], fp32, tag="t")
        nc.tensor.transpose(ps_h0s[:], h0s_s[:], ident[:B, :B])
        h0sT = sb.tile([GS, B], fp32, tag="h0sT")
        nc.vector.tensor_copy(h0sT[:], ps_h0s[:])

        # combined rz bias = bih + bhh (gates r,z use the summed bias)
        brz = sb.tile([1, 2 * GS], fp32, tag="brz")
        nc.vector.tensor_add(brz[:], bih_s[:, :2 * GS], bhh_s[:, :2 * GS])

        # ---------- phase 2: GRU (sharded gates, all on partitions 0..63) ----------
        ps_r = psM.tile([GS, B], fp32, tag="m")
        ps_z = psM.tile([GS, B], fp32, tag="m")
        ps_gin = psM.tile([GS, B], fp32, tag="m")
        ps_ghn = psM.tile([GS, B], fp32, tag="m")
        for g, ps in ((0, ps_r), (1, ps_z)):
            for k in range(4):
                nc.tensor.matmul(ps[:], wih_s[:, k, g * GS:(g + 1) * GS], xT[:, k, :],
                                 start=(k == 0), stop=False)
            for k in range(4):
                nc.tensor.matmul(ps[:], whh_s[:, k, g * GS:(g + 1) * GS], h0T[:, k, :],
                                 start=False, stop=False)
            nc.tensor.matmul(ps[:], brz[:, g * GS:(g + 1) * GS], ones10[:],
                             start=False, stop=True)
        for w, bias, ps in ((wih_s, bih_s, ps_gin), (whh_s, bhh_s, ps_ghn)):
            src = xT if w is wih_s else h0T
            for k in range(4):
                nc.tensor.matmul(ps[:], w[:, k, 2 * GS:3 * GS], src[:, k, :],
                                 start=(k == 0), stop=False)
            nc.tensor.matmul(ps[:], bias[:, 2 * GS:3 * GS], ones10[:],
                             start=False, stop=True)

        r_sb = sb.tile([GS, B], fp32, tag="r")
        z_sb = sb.tile([GS, B], fp32, tag="z")
        nc.scalar.activation(r_sb[:], ps_r[:], AF.Sigmoid)
        nc.scalar.activation(z_sb[:], ps_z[:], AF.Sigmoid)
        tmpn = sb.tile([GS, B], fp32, tag="tmpn")
        nc.vector.tensor_mul(tmpn[:], r_sb[:], ps_ghn[:])
        nc.vector.tensor_add(tmpn[:], tmpn[:], ps_gin[:])
        n_sb = sb.tile([GS, B], fp32, tag="n")
        nc.scalar.activation(n_sb[:], tmpn[:], AF.Tanh)
        zn = sb.tile([GS, B], fp32, tag="zn")
        nc.vector.tensor_mul(zn[:], z_sb[:], n_sb[:])
        zh = sb.tile([GS, B], fp32, tag="zh")
        nc.vector.tensor_mul(zh[:], z_sb[:], h0sT[:])
        h1c = sb.tile([GS, B], fp32, tag="h1c")
        nc.vector.tensor_sub(h1c[:], n_sb[:], zn[:])
        nc.vector.tensor_add(h1c[:], h1c[:], zh[:])

        # AllGather h1
        nc.sync.dma_start(h1cc[:], h1c[:])
        h1Ts = sb.tile([128, 4, B], fp32, tag="h1Ts")
        if os.environ.get("K_NOCOLL"):
            nc.vector.memset(h1Ts[:], 0.125)
            nc.sync.dma_start(h1Ts[:GS, 0, :], h1cc[:])
            nc.sync.dma_start(h1T_o[:GS, :], h1cc[:])
        else:
            nc.gpsimd.collective_compute("AllGather", mybir.AluOpType.bypass,
                                         replica_groups=groups,
                                         ins=[h1cc[:]], outs=[h1all[:]])
            nc.sync.dma_start(h1Ts[:], h1all.rearrange("(j p) b -> p j b", p=128))
            nc.sync.dma_start(h1T_o[:], h1all[:])

        # ---------- phase 3: attention ----------
        # pre-transpose E chunks: ET[(b,j)] = (128 h-sub, 64 l)
        ET = sb.tile([128, 40, LS], fp32, tag="ET")
        for ci in range(40):
            ps = psT.tile([128, LS], fp32, tag="t")
            nc.tensor.transpose(ps[:], E[:, ci * 128:(ci + 1) * 128],
                                ident[:LS, :LS])
            cp(ci % 2, ET[:, ci, :], ps[:])

        ps_en = psM.tile([LS, B], fp32, tag="m")
        for b in range(B):
            for j in range(4):
                nc.tensor.matmul(ps_en[:, b:b + 1], ET[:, b * 4 + j, :],
                                 h1Ts[:, j, b:b + 1],
                                 start=(j == 0), stop=(j == 3))
        en_lb = sb.tile([LS, B], fp32, tag="en_lb")
        nc.vector.tensor_copy(en_lb[:], ps_en[:])

        mx = sb.tile([LS, 1], fp32, tag="mx")
        nc.vector.reduce_max(out=mx[:], in_=en_lb[:], axis=mybir.AxisListType.X)
        nmx = sb.tile([LS, 1], fp32, tag="nmx")
        nc.vector.tensor_scalar_mul(nmx[:], mx[:], -1.0)
        ex = sb.tile([LS, B], fp32, tag="ex")
        nc.scalar.activation(ex[:], en_lb[:], AF.Exp, bias=nmx[:])
        sm = sb.tile([LS, 1], fp32, tag="sm")
        nc.vector.reduce_sum(out=sm[:], in_=ex[:], axis=mybir.AxisListType.X)
        rs = sb.tile([LS, 1], fp32, tag="rs")
        nc.vector.reciprocal(rs[:], sm[:])
        attn = sb.tile([LS, B], fp32, tag="attn")
        nc.vector.tensor_scalar_mul(attn[:], ex[:], rs[:])
        nc.sync.dma_start(attnw[:], attn[:])

        adiag = sb.tile([LS, B * B], fp32, tag="adiag")
        nc.vector.memset(adiag[:], 0.0)
        for b in range(B):
            nc.vector.tensor_copy(adiag[:, b * B + b:b * B + b + 1],
                                  attn[:, b:b + 1])
        ps_ctx = psM.tile([B, H], fp32, tag="m")
        for b in range(B):
            nc.tensor.matmul(ps_ctx[:], adiag[:, b * B:(b + 1) * B],
                             E[:, b * H:(b + 1) * H],
                             start=(b == 0), stop=(b == B - 1))
        ctx_sb = sb.tile([B, H], fp32, tag="ctx_sb")
        nc.vector.tensor_copy(ctx_sb[:], ps_ctx[:])
        nc.sync.dma_start(ctxcc[:], ctx_sb[:])
        ctxf = sb.tile([B, H], fp32, tag="ctxf")
        if os.environ.get("K_NOCOLL"):
            nc.sync.dma_start(ctxf[:], ctxcc[:])
            nc.sync.dma_start(ctx_o[:], ctxcc[:])
        else:
            nc.gpsimd.collective_compute("AllReduce", mybir.AluOpType.add,
                                         replica_groups=groups,
                                         ins=[ctxcc[:]], outs=[ctxall[:]])
            nc.sync.dma_start(ctxf[:], ctxall[:])
            nc.sync.dma_start(ctx_o[:], ctxall[:])

        ctxT = sb.tile([128, 4, B], fp32, tag="ctxT")
        for j in range(4):
            ps = psT.tile([128, B], fp32, tag="t")
            nc.tensor.transpose(ps[:], ctxf[:, j * 128:(j + 1) * 128],
                                ident[:B, :B])
            cp(j % 2, ctxT[:, j, :], ps[:])

        # ---------- phase 4: concat projection (tanh) ----------
        coT = sb.tile([128, 4, B], fp32, tag="coT")
        for m in range(4):
            ps = psM.tile([128, B], fp32, tag="m")
            for k in range(8):
                rhs = h1Ts[:, k, :] if k < 4 else ctxT[:, k - 4, :]
                nc.tensor.matmul(ps[:], wcat_s[:, k, m * 128:(m + 1) * 128], rhs,
                                 start=(k == 0), stop=(k == 7))
            nc.scalar.activation(coT[:, m, :], ps[:], AF.Tanh,
                                 bias=bcat_s[:, m:m + 1])

        # ---------- phase 5: vocab projection ----------
        for ns in range(8):
            n0 = ns * 512
            nn = min(VS, n0 + 512) - n0
            ps = psM.tile([B, 512], fp32, tag="m")
            for k in range(4):
                nc.tensor.matmul(ps[:, :nn], coT[:, k, :], wvo_t[ns][:, k, :nn],
                                 start=(k == 0), stop=False)
            nc.tensor.matmul(ps[:, :nn], ones10[:], bvo_s[:, n0:n0 + nn],
                             start=False, stop=True)
            lg = sb.tile([B, 512], fp32, tag=f"lg{ns % 2}")
            cp(ns % 2, lg[:, :nn], ps[:, :nn])
            nc.sync.dma_start(logits[:, n0:n0 + nn], lg[:, :nn])

        # ---------- phase 6: kb sum + writeback ----------
        e2sum = sb.tile([128, 5, H], fp32, tag="e2sum")
        nc.vector.tensor_add(e2sum[:], kbg[:, 0:5, :], kbg[:, 5:10, :])
        nc.vector.tensor_add(e2sum[:], e2sum[:], kbg[:, 10:15, :])
        nc.sync.dma_start(e2c.rearrange("(cc p) h -> p cc h", p=128), e2sum[:])

    nc.compile()
    return nc


def _get_program():
    if "nc" not in _CACHE:
        _CACHE["nc"] = _build_program()
    return _CACHE["nc"]


def _j_range(c):
    lo = (c * SLAB) // H
    hi = -((-(c + 1) * SLAB) // H)  # ceil
    return lo, hi


def _prep_inputs(input_seq, kb_inputs, last_context, last_hidden, encoder_outputs,
                 emb, emb_kb, w_ih, w_hh, b_ih, b_hh, w_concat, b_concat,
                 w_out, b_out):
    f = np.float32
    x = np.ascontiguousarray(emb[np.asarray(input_seq).astype(np.int64)], dtype=f)
    h0 = np.ascontiguousarray(last_hidden[0], dtype=f)
    xh = np.stack([x, h0]).astype(f)  # (2, B, H)
    wcat = np.ascontiguousarray(np.asarray(w_concat, dtype=f).T)
    embkb = np.ascontiguousarray(emb_kb, dtype=f)
    enc = np.asarray(encoder_outputs, dtype=f)
    kbi = np.asarray(kb_inputs).astype(np.int64)

    in_maps = []
    for c in range(NCORES):
        rows = np.r_[c * GS:(c + 1) * GS, H + c * GS:H + (c + 1) * GS,
                     2 * H + c * GS:2 * H + (c + 1) * GS]
        lo, hi = _j_range(c)
        jc = hi - lo
        idx = np.zeros((3, B, NJ), np.int16)
        idx[:, :, :jc] = kbi[:, lo:hi, :].transpose(2, 0, 1)
        idx_w = np.tile(idx.reshape(NIDX // 16, 16).T, (8, 1))  # (128, NIDX//16)
        m = {
            "xh": xh,
            "h0s": np.ascontiguousarray(h0[:, c * GS:(c + 1) * GS]),
            "wih": np.ascontiguousarray(np.asarray(w_ih, f)[rows].T),
            "whh": np.ascontiguousarray(np.asarray(w_hh, f)[rows].T),
            "bih": np.ascontiguousarray(np.asarray(b_ih, f)[rows][None]),
            "bhh": np.ascontiguousarray(np.asarray(b_hh, f)[rows][None]),
            "encs": np.ascontiguousarray(enc[c * LS:(c + 1) * LS]),
            "wcat": wcat,
            "bcat": np.asarray(b_concat, f),
            "wvo": np.ascontiguousarray(np.asarray(w_out, f)[c * VS:(c + 1) * VS].T),
            "bvo": np.ascontiguousarray(np.asarray(b_out, f)[c * VS:(c + 1) * VS][None]),
            "kbidx": np.ascontiguousarray(idx_w),
            "embkb": embkb,
        }
        in_maps.append(m)
    return in_maps


def _assemble(results):
    f = np.float32
    out = np.concatenate([results[c]["logits"] for c in range(NCORES)], axis=1)
    h1 = np.ascontiguousarray(results[0]["h1T"].T)[None]          # (1, B, H)
    context = results[0]["ctxo"]
    attn_full = np.concatenate([results[c]["attnw"] for c in range(NCORES)], axis=0)
    attn_weights = np.ascontiguousarray(attn_full.T)[:, None, :]   # (B, 1, L)
    kb_attn = np.zeros((B, H, KB_PAD + KB), f)
    for c in range(NCORES):
        lo, hi = _j_range(c)
        jc = hi - lo
        off = c * SLAB - lo * H
        e2 = results[c]["e2c"].reshape(B, NJ, H)
        for b in range(B):
            seg = e2[b, :jc].reshape(-1)[off:off + SLAB]
            kb_attn[b, c * LS:(c + 1) * LS, KB_PAD:] = seg.reshape(LS, KB)
    return (np.ascontiguousarray(out), np.ascontiguousarray(context),
            h1, attn_weights, kb_attn)


def run_sim(**inputs):
    """Run via the multi-core interpreter (correctness check, no HW)."""
    from concourse.bass_interp import MultiCoreSim
    nc = _get_program()
    in_maps = _prep_inputs(**inputs)
    sim = MultiCoreSim(nc, num_cores=NCORES, trace=False)
    for c in range(NCORES):
        for k, v in in_maps[c].items():
            sim.cores[c].tensor(k)[:] = v
    sim.simulate()
    results = [{k: np.array(sim.cores[c].tensor(k))
                for k in ("logits", "attnw", "h1T", "ctxo", "e2c")}
               for c in range(NCORES)]
    return _assemble(results)


def kernel(**inputs):
    from concourse.bass_utils import run_bass_kernel_spmd
    nc = _get_program()
    in_maps = _prep_inputs(**inputs)
    res = run_bass_kernel_spmd(nc, in_maps, list(range(NCORES)))
    return _assemble(res.results)


def kernel_profiled(**inputs):
    from concourse.bass_utils import run_bass_kernel_spmd
    nc = _get_program()
    in_maps = _prep_inputs(**inputs)
    res = run_bass_kernel_spmd(nc, in_maps, list(range(NCORES)), trace=True)
    return _assemble(res.results), res
